# revision 2
# baseline (speedup 1.0000x reference)
"""Feature-pyramid ROIAlign (multi-level crop) on 8 TRN2 NeuronCores — v5.

Host routes (level assignment, 11x11 windows, bf16 bilinear weight
matrices); the device gathers patch cells via InstDMAGatherAnt (one
gather per class-run per group of up to 8 proposals — ~1us GpSimd issue
for 8 patches instead of 8 x ~0.6-1.1us per-patch DMA issues), then
interpolates with two k=121 bf16 matmuls per proposal into PSUM and
writes the c-major f32->bf16 output shard.

dma_gather semantics: ordinal o -> partition o%128, block o//128; idx
table wrapped [16, cols] (ordinal o at partition o%16, col o//16),
replicated to 128 partitions. idx values are int16 -> the channels-last
arena is addressed via <=32768-row slices (f0 needs 3 overlapping
slices; proposals are classed by (level, slice) and sharded per class so
all 8 cores share one SPMD graph).

Group sizes ramp 2,2,4,4,...8...,4,2,2 so the first matmul isn't gated
on a 380KB wmat DMA (22.5GB/s per-queue cap) and the last out-write
tail is short.
"""
import os
import numpy as np
import ml_dtypes

RPN_SCALES = (2.0, 4.0, 8.0, 16.0)
BASE_SIZES = (8.0, 16.0, 32.0, 64.0)
S = 14
S2 = S * S
PW = 11
CELLS = PW * PW
C = 256
MAP_HW = (256, 128, 64, 32)
ARENA_BASE = (0, 65536, 81920, 86016)  # cell-row base of each level
ARENA_ROWS = 87040
N_CORES = 8

# classes: (level, slice_start, slice_end) in arena-absolute cell rows.
# f0 (65536 rows) exceeds int16 gather indexing -> 3 overlapping slices.
CLASS_SLICE = (
    (0, 0, 32768),      # lvl0, window base row r0 < 29952
    (0, 29696, 62464),  # lvl0, 29952 <= r0 < 59648
    (0, 59392, 65536),  # lvl0, r0 >= 59648
    (1, 65536, 81920),
    (2, 81920, 86016),
    (3, 86016, 87040),
)
N_CLASSES = len(CLASS_SLICE)

LAST_EXEC_TIME_NS = None
_GRAPH_CACHE = {}


def _route_and_weights(proposals):
    p = proposals.astype(np.float32)
    x0, y0, x1, y1 = p[:, 1], p[:, 2], p[:, 3], p[:, 4]
    sizes = np.sqrt((x1 - x0) * (y1 - y0))
    base = np.asarray(BASE_SIZES, dtype=np.float32)
    dist = np.abs(sizes[:, None] - base[None, :])
    lvl = np.argmin(dist, axis=1).astype(np.int32)

    N = p.shape[0]
    stride = np.asarray(RPN_SCALES, dtype=np.float32)[lvl]
    M = np.asarray(MAP_HW, dtype=np.int32)[lvl]

    fx0, fy0, fx1, fy1 = (c / stride for c in (x0, y0, x1, y1))
    bw = (fx1 - fx0) / np.float32(S)
    bh = (fy1 - fy0) / np.float32(S)
    grid = np.arange(S, dtype=np.float32) + np.float32(0.5)
    xs = fx0[:, None] + grid[None, :] * bw[:, None] - np.float32(0.5)
    ys = fy0[:, None] + grid[None, :] * bh[:, None] - np.float32(0.5)

    def split(coord, Mv):
        c0 = np.floor(coord)
        frac = coord - c0
        i0 = np.clip(c0.astype(np.int64), 0, Mv - 1).astype(np.int32)
        i1 = np.minimum(i0 + 1, Mv - 1).astype(np.int32)
        return i0, i1, frac.astype(np.float32)

    Mv = M[:, None]
    yi0, yi1, wy = split(ys, Mv)
    xi0, xi1, wx = split(xs, Mv)

    oy = np.clip(yi0.min(axis=1), 0, M - PW)
    ox = np.clip(xi0.min(axis=1), 0, M - PW)
    ly0, ly1 = yi0 - oy[:, None], yi1 - oy[:, None]
    lx0, lx1 = xi0 - ox[:, None], xi1 - ox[:, None]
    assert ly0.min() >= 0 and lx0.min() >= 0 and ly1.max() < PW and lx1.max() < PW, \
        "proposal spans >11 feature cells; patch window too small"

    ii = np.arange(S)
    nn = np.arange(N)[:, None]
    Wy = np.zeros((N, S, PW), dtype=np.float32)
    Wx = np.zeros((N, S, PW), dtype=np.float32)
    np.add.at(Wy, (nn, ii[None, :], ly0), 1.0 - wy)
    np.add.at(Wy, (nn, ii[None, :], ly1), wy)
    np.add.at(Wx, (nn, ii[None, :], lx0), 1.0 - wx)
    np.add.at(Wx, (nn, ii[None, :], lx1), wx)
    Wfull = np.einsum("niy,njx->nyxij", Wy, Wx).reshape(N, CELLS, S2)

    # class + slice-relative window base row for the gather index table
    r0_abs = np.asarray(ARENA_BASE)[lvl] + oy.astype(np.int64) * M + ox
    r0_lvl = oy.astype(np.int64) * M + ox
    cls = np.where(lvl == 0,
                   np.where(r0_lvl < 29952, 0, np.where(r0_lvl < 59648, 1, 2)),
                   lvl + 2).astype(np.int32)
    slice_start = np.asarray([cs[1] for cs in CLASS_SLICE], dtype=np.int64)[cls]
    base_rel = (r0_abs - slice_start).astype(np.int32)
    assert base_rel.min() >= 0 and (base_rel + (PW - 1) * M + PW - 1).max() < 32768
    return lvl, cls, base_rel, Wfull


def _shard(cls):
    """Distribute proposals to cores balanced per class so every core has
    the same class sequence (one SPMD graph). Returns slot gids [8, M] and
    the class id per slot."""
    per_core = [[] for _ in range(N_CORES)]
    class_seq = []
    counts = []
    ids_by_class = []
    for c in range(N_CLASSES):
        ids = np.where(cls == c)[0]
        ids_by_class.append(ids)
        if len(ids) == 0:
            counts.append(0)
            continue
        pad = (-len(ids)) % N_CORES
        counts.append(len(ids) + pad)
    M = sum(ct // N_CORES for ct in counts)
    if M % 2 == 1:  # pairing needs an even slot count
        c = int(np.argmax(counts))
        counts[c] += N_CORES
    for c in range(N_CLASSES):
        ids = ids_by_class[c]
        if counts[c] == 0:
            continue
        pad = counts[c] - len(ids)
        ids = np.concatenate([ids, np.repeat(ids[-1], pad)])
        per = len(ids) // N_CORES
        for k in range(N_CORES):
            per_core[k].extend(ids[k::N_CORES].tolist())
        class_seq.extend([c] * per)
    return (np.asarray(per_core, dtype=np.int64),
            np.asarray(class_seq, dtype=np.int64))


def _group_sizes(M):
    start, end = [2, 2, 4, 4], [4, 2, 2]
    mid = M - sum(start) - sum(end)
    assert mid >= 0 and mid % 2 == 0
    n8, rem = divmod(mid, 8)
    return start + [8] * n8 + ([rem] if rem else []) + end


def _build_graph(class_seq, group_sizes):
    import concourse.bass as bass  # noqa: F401
    import concourse.bacc as bacc
    import concourse.mybir as mybir
    import concourse.tile as tile

    M = len(class_seq)
    nc = bacc.Bacc()
    arena = nc.declare_dram_parameter("arena", [ARENA_ROWS, C],
                                      mybir.dt.bfloat16, isOutput=False)
    wmat = nc.declare_dram_parameter("wmat", [CELLS, M, S2], mybir.dt.bfloat16,
                                     isOutput=False)
    idxs = nc.declare_dram_parameter("idxs", [128, M * 8], mybir.dt.int16,
                                     isOutput=False)
    out = nc.declare_dram_parameter("out", [C, M, S2], mybir.dt.bfloat16,
                                    isOutput=True)

    n_groups = len(group_sizes)
    # idx chunk boundaries (group-aligned, ~4 chunks)
    bnds = [0]
    acc = 0
    tgt = (M + 3) // 4
    for ns in group_sizes:
        acc += ns
        if acc - bnds[-1] >= tgt and acc < M:
            bnds.append(acc)
    bnds.append(M)

    with tile.TileContext(nc) as tc:
        with (
            tc.tile_pool(name="pidx", bufs=len(bnds) - 1) as pidx,
            tc.tile_pool(name="wpool", bufs=4) as pwp,
            tc.tile_pool(name="gpool", bufs=4) as pg,
            tc.tile_pool(name="outp", bufs=4) as po,
            tc.tile_pool(name="ps", bufs=4, space="PSUM") as ppsum,
        ):
            chunk_tiles = []
            for ci in range(len(bnds) - 1):
                c0, c1 = bnds[ci] * 8, bnds[ci + 1] * 8
                it = pidx.tile([128, c1 - c0], mybir.dt.int16)
                eng = nc.sync if ci % 2 == 0 else nc.scalar
                eng.dma_start(it[:], idxs[:, c0:c1])
                chunk_tiles.append((bnds[ci], bnds[ci + 1], it))

            def idx_slice(ra, rb):
                for (g0, g1, it) in chunk_tiles:
                    if ra >= g0 and rb <= g1:
                        return it[:, (ra - g0) * 8:(rb - g0) * 8]
                raise AssertionError("idx run crosses chunk boundary")

            a = 0
            for ns in group_sizes:
                b = a + ns
                wt = pwp.tile([CELLS, ns * S2], mybir.dt.bfloat16, tag="wt")
                nc.sync.dma_start(
                    wt[:].rearrange("k (p n) -> k p n", p=ns),
                    wmat[:, a:b, :])
                gp = pg.tile([128, ns * 256], mybir.dt.bfloat16, tag="gp")
                # one gather per same-class run within the group
                ra = a
                while ra < b:
                    rb = ra
                    while rb < b and class_seq[rb] == class_seq[ra]:
                        rb += 1
                    _, s0, s1 = CLASS_SLICE[class_seq[ra]]
                    nc.gpsimd.dma_gather(
                        out_ap=gp[:, (ra - a) * 256:(rb - a) * 256].rearrange(
                            "p (w c) -> p w c", c=256),
                        in_ap=arena[s0:s1, :],
                        idxs_ap=idx_slice(ra, rb),
                        num_idxs=(rb - ra) * 128,
                        num_idxs_reg=(rb - ra) * 128,
                        elem_size=256,
                    )
                    ra = rb
                outAB = po.tile([128, 2 * ns * S2], mybir.dt.bfloat16,
                                tag="outAB")
                for q0 in range(0, ns, 2):
                    # one 2-bank PSUM tile per slot pair: A halves in bank 0
                    # (cols 0..392), B halves in bank 1 (cols 512..904)
                    psAB = ppsum.tile([128, 1024], mybir.dt.float32, tag="psAB")
                    for dq in range(2):
                        q = q0 + dq
                        rhs = wt[:, q * S2:(q + 1) * S2]
                        nc.tensor.matmul(psAB[:, dq * S2:(dq + 1) * S2],
                                         gp[0:CELLS, q * 256:q * 256 + 128],
                                         rhs, start=True, stop=True)
                        nc.tensor.matmul(psAB[:, 512 + dq * S2:512 + (dq + 1) * S2],
                                         gp[0:CELLS, q * 256 + 128:(q + 1) * 256],
                                         rhs, start=True, stop=True)
                    src = psAB[:].rearrange("p (b n) -> p b n", b=2)[:, :, 0:2 * S2]
                    dst = outAB[:].rearrange("p (b n) -> p b n", b=2)[
                        :, :, q0 * S2:(q0 + 2) * S2]
                    nc.vector.tensor_copy(dst, src)
                nc.sync.dma_start(out[0:128, a:b, :], outAB[:, 0:ns * S2])
                nc.scalar.dma_start(out[128:256, a:b, :],
                                    outAB[:, ns * S2:2 * ns * S2])
                a = b
    nc.finalize()
    return nc


def _prep_core_inputs(k, slot_gid, lvl, base_rel, Wbf):
    M = slot_gid.shape[1]
    gids = slot_gid[k]
    wm = np.ascontiguousarray(Wbf[gids].transpose(1, 0, 2))  # [121, M, 196]

    W = np.asarray(MAP_HW)[lvl[gids]].astype(np.int32)      # [M]
    dy = np.repeat(np.arange(PW, dtype=np.int32), PW)
    dx = np.tile(np.arange(PW, dtype=np.int32), PW)
    cells = base_rel[gids][:, None] + dy[None, :] * W[:, None] + dx[None, :]
    idx_flat = np.zeros((M, 128), np.int16)
    idx_flat[:, :CELLS] = cells.astype(np.int16)
    # wrap: ordinal o -> partition o%16, col o//16; replicate to 128 rows
    wrapped = idx_flat.reshape(-1, 16).T                      # [16, M*8]
    idx_t = np.ascontiguousarray(np.tile(wrapped, (8, 1)))    # [128, M*8]
    return wm, idx_t


def _install_profile_hook():
    """Register the NTFF profile hook (ctypes into libaxon_pjrt.so) so
    run_bass_kernel_spmd(trace=True) can report exec_time_ns under axon.
    No-op if already present or the .so lacks the symbols."""
    import contextlib
    import ctypes
    import sys
    import types
    if "antenv.axon_hooks" in sys.modules:
        return
    so_path = "/opt/axon/libaxon_pjrt.so"
    try:
        lib = ctypes.CDLL(so_path)
        lib.axon_start_nrt_profile.argtypes = [
            ctypes.POINTER(ctypes.c_int64), ctypes.c_size_t]
        lib.axon_start_nrt_profile.restype = ctypes.c_int64
        lib.axon_stop_nrt_profile.argtypes = [ctypes.c_char_p]
        lib.axon_stop_nrt_profile.restype = ctypes.c_int64
    except (OSError, AttributeError):
        return

    @contextlib.contextmanager
    def _hook(output_dir, device_ids):
        import jax
        jax.devices()
        if device_ids:
            ids = (ctypes.c_int64 * len(device_ids))(*device_ids)
            rc = lib.axon_start_nrt_profile(ids, len(device_ids))
        else:
            rc = lib.axon_start_nrt_profile(None, 0)
        if rc != 0:
            raise RuntimeError(f"axon_start_nrt_profile rc={rc}")
        try:
            yield
        finally:
            n = lib.axon_stop_nrt_profile(str(output_dir).encode())
            if n < 0:
                raise RuntimeError(f"axon_stop_nrt_profile rc={n}")

    mod = types.ModuleType("antenv.axon_hooks")
    mod.get_axon_ntff_profile_hook = lambda: _hook
    mod.set_axon_ntff_profile_hook = lambda h: None
    sys.modules["antenv.axon_hooks"] = mod
    try:
        import antenv
        antenv.axon_hooks = mod
    except ImportError:
        pass


def kernel(f0, f1, f2, f3, proposals):
    global LAST_EXEC_TIME_NS
    try:
        _install_profile_hook()
    except Exception:
        pass
    from concourse.bass_utils import run_bass_kernel_spmd

    feats = (f0, f1, f2, f3)
    N = proposals.shape[0]
    lvl, cls, base_rel, Wfull = _route_and_weights(np.asarray(proposals))
    slot_gid, class_seq = _shard(cls)
    M = slot_gid.shape[1]
    group_sizes = _group_sizes(M)

    key = tuple(class_seq.tolist())
    if key not in _GRAPH_CACHE:
        _GRAPH_CACHE[key] = _build_graph(class_seq, group_sizes)
    nc = _GRAPH_CACHE[key]

    arena_np = np.concatenate([
        np.ascontiguousarray(np.asarray(f)[0].transpose(1, 2, 0)).astype(
            ml_dtypes.bfloat16).reshape(-1, C)
        for f in feats
    ], axis=0)
    assert arena_np.shape[0] == ARENA_ROWS
    Wbf = Wfull.astype(ml_dtypes.bfloat16)

    in_maps = []
    for k in range(N_CORES):
        wm, idx_t = _prep_core_inputs(k, slot_gid, lvl, base_rel, Wbf)
        in_maps.append({"arena": arena_np, "wmat": wm, "idxs": idx_t})

    trace = os.environ.get("KERNEL_TRACE", "0") == "1"
    res = run_bass_kernel_spmd(nc, in_maps, list(range(N_CORES)), trace=trace)
    LAST_EXEC_TIME_NS = res.exec_time_ns

    out_full = np.zeros((N, C, S2), dtype=np.float32)
    for k in range(N_CORES):
        out_full[slot_gid[k]] = res.results[k]["out"].astype(np.float32).transpose(1, 0, 2)
    return out_full.reshape(N, C, S, S)
